# revision 56
# baseline (speedup 1.0000x reference)
"""Trainium2 Bass kernel for nn_DongTaiBaGuaZhen (8-core SPMD).

Sharding: core c handles (batch b = c//2, sequence half c%2) -> 2048 tokens.
The tiny cross-head impedance path (B*H*H MLP, ~0.01% of FLOPs) depends only
on mean_s(x) because the head projection is linear; it is computed on the
host and folded into a per-batch weight matrix W2_b so the device work is
three dense 1024x1024 matmuls per token chunk plus layernorm:

    heads_T = WpT.T @ x_T            (cos-modulated projection, folded on host)
    y_T     = OWT.T @ heads_T + scale(s) * (W2T.T @ heads_T) + x_T + out_b
    out_T   = LN(y_T) * g + b        (stats via PE ones-reductions)

Everything on device runs in transposed layout [features, tokens] so all
matmul contractions live on the partition axis; fp32r matmuls (1 cyc/row).
"""

import math
from contextlib import ExitStack

import numpy as np

B, S, D, H, P = 4, 4096, 1024, 8, 32
K = D // H           # 128
NCORES = 8
SC = S // 2          # tokens per core = 2048
CH = 256             # token chunk
NCH = SC // CH       # 8
PT = 128             # partitions
NT = D // PT         # 8 feature tiles

_CACHE = {}


def _erf(v):
    try:
        from scipy.special import erf
        return erf(v)
    except ImportError:
        return np.vectorize(math.erf)(v)


def _host_impedance(x, W_proj, resonance_freqs, pol_W, pol_b, imp_W1, imp_b1,
                    imp_W2, imp_b2):
    """Impedance path in float64 on host. Returns (impedance f32 (B,H,H), coef f64)."""
    f8 = np.float64
    cosv = np.cos(resonance_freqs.astype(f8) * np.pi)            # (H, K)
    xbar = x.mean(axis=1, dtype=f8)                              # (B, D)
    summary = np.einsum("bd,hkd->bhk", xbar, W_proj.astype(f8)) * cosv[None]
    pol = np.tanh(np.einsum("bhk,hpk->bhp", summary, pol_W.astype(f8))
                  + pol_b.astype(f8)[None])
    pol = pol / np.maximum(np.linalg.norm(pol, axis=-1, keepdims=True), 1e-12)
    dots = np.einsum("bhp,bgp->bhg", pol, pol)
    z = dots[..., None]
    pre = z * imp_W1.astype(f8)[:, 0] + imp_b1.astype(f8)
    hid = 0.5 * pre * (1.0 + _erf(pre / math.sqrt(2.0)))         # exact gelu
    imp = np.einsum("bijc,oc->bijo", hid, imp_W2.astype(f8))[..., 0] \
        + imp_b2.astype(f8)[0]
    imp = np.logaddexp(imp, 0.0)                                 # softplus
    eye = np.eye(H)
    impedance = imp * (1.0 - eye)
    coef = (0.1 / (1.0 + impedance)) * (1.0 - eye)
    return impedance.astype(np.float32), coef


def _build_program(flags=(True, True)):
    import concourse.bass as bass
    import concourse.bacc as bacc
    import concourse.tile as tile
    from concourse import mybir

    f32 = mybir.dt.float32
    f32r = mybir.dt.float32r
    bf16 = mybir.dt.bfloat16
    AF = mybir.ActivationFunctionType

    nc = bacc.Bacc()

    # matmul operands live as float32r end-to-end (fp32 bits; PE rounds)
    xT_d = nc.declare_dram_parameter("xT", [D, SC], f32r, isOutput=False)
    wp_d = nc.declare_dram_parameter("wpT", [D, D], f32r, isOutput=False)
    ow_d = nc.declare_dram_parameter("owT", [D, D], f32r, isOutput=False)
    w2_d = nc.declare_dram_parameter("w2T", [D, D], bf16, isOutput=False)
    ones_d = nc.declare_dram_parameter("ones_col", [PT, 1], f32r, isOutput=False)
    onesr_d = nc.declare_dram_parameter("ones_row", [1, PT], f32r, isOutput=False)
    sr_d = nc.declare_dram_parameter("scale_row", [SC], f32, isOutput=False)
    lng_d = nc.declare_dram_parameter("lng", [PT, NT], f32, isOutput=False)
    lnb_d = nc.declare_dram_parameter("lnb", [PT, NT], f32, isOutput=False)
    outb_d = nc.declare_dram_parameter("outb", [PT, NT], f32, isOutput=False)
    out_d = nc.declare_dram_parameter("outT", [D, SC], f32, isOutput=True)

    trivial_outb, trivial_ln = flags

    with tile.TileContext(nc) as tc, ExitStack() as ctx:
        consts = ctx.enter_context(tc.tile_pool(name="consts", bufs=1))
        wpool = ctx.enter_context(tc.tile_pool(name="weights", bufs=1))
        xt_pool = ctx.enter_context(tc.tile_pool(name="xt", bufs=16))
        heads_pool = ctx.enter_context(tc.tile_pool(name="heads", bufs=12))
        headsb_pool = ctx.enter_context(tc.tile_pool(name="headsb", bufs=12))
        ybig_pool = ctx.enter_context(tc.tile_pool(name="ybig", bufs=16))
        tuv_pool = ctx.enter_context(tc.tile_pool(name="tuv", bufs=4))
        outp_pool = ctx.enter_context(tc.tile_pool(name="outp", bufs=6))
        bc_pool = ctx.enter_context(tc.tile_pool(name="bc", bufs=2))
        row_pool = ctx.enter_context(tc.tile_pool(name="rows", bufs=4))
        ps_h = ctx.enter_context(tc.tile_pool(name="ps_h", bufs=2, space="PSUM"))
        ps_a = ctx.enter_context(tc.tile_pool(name="ps_a", bufs=2, space="PSUM"))
        ps_b = ctx.enter_context(tc.tile_pool(name="ps_b", bufs=2, space="PSUM"))
        ps_st = ctx.enter_context(tc.tile_pool(name="ps_st", bufs=1, space="PSUM"))
        ps_bc = ctx.enter_context(tc.tile_pool(name="ps_bc", bufs=1, space="PSUM"))

        # ---- DMA queue load-balancing: the two HWDGE queues (sync, scalar)
        # each sustain a limited rate, so alternate every large transfer
        # between them, emitted in priority order.
        _qs = [nc.sync, nc.scalar]
        _qi = [0]

        def qdma(**kw):
            eng = _qs[_qi[0] % len(_qs)]
            _qi[0] += 1
            eng.dma_start(**kw)

        xt_store = {}

        def load_xt_pair(p):
            tiles = []
            pcols = slice(2 * p * CH, (2 * p + 2) * CH)
            for dt_ in range(NT):
                xt = xt_pool.tile([PT, 2 * CH], f32r)
                qdma(out=xt[:], in_=xT_d[dt_ * PT:(dt_ + 1) * PT, pcols])
                tiles.append(xt)
            xt_store[2 * p] = [t[:, 0:CH] for t in tiles]
            xt_store[2 * p + 1] = [t[:, CH:2 * CH] for t in tiles]

        # startup: wp + x(pair 0) first, interleaved across both queues
        wp_sb = []
        for i in range(NT):
            t = wpool.tile([PT, D], f32r, tag=f"wp{i}")
            qdma(out=t[:], in_=wp_d[i * PT:(i + 1) * PT, :])
            wp_sb.append(t)
        load_xt_pair(0)

        ones_col = consts.tile([PT, 1], f32r, tag="ones_col")
        nc.sync.dma_start(out=ones_col[:], in_=ones_d[:, :])
        ones_row = consts.tile([1, PT], f32r, tag="ones_row")
        nc.sync.dma_start(out=ones_row[:], in_=onesr_d[:, :])
        eps_sb = consts.tile([1, 1], f32, tag="eps")
        nc.vector.memset(eps_sb[:], 1e-5)
        # scale broadcast tile [128, SC] via partition-step-0 DMA from DRAM
        scale_bc = consts.tile([PT, SC], f32, tag="scale_bc")
        srow_ap = sr_d[:]
        bcast_in = bass.AP(tensor=srow_ap.tensor, offset=srow_ap.offset,
                           ap=[[0, PT]] + list(srow_ap.ap))
        nc.gpsimd.dma_start(out=scale_bc[:], in_=bcast_in)

        ow_sb, w2_sb = [], []
        for i in range(NT):
            t = wpool.tile([PT, D], f32r, tag=f"ow{i}")
            qdma(out=t[:], in_=ow_d[i * PT:(i + 1) * PT, :])
            ow_sb.append(t)
        for i in range(NT):
            t = wpool.tile([PT, D], bf16, tag=f"w2{i}")
            qdma(out=t[:], in_=w2_d[i * PT:(i + 1) * PT, :])
            w2_sb.append(t)

        if not trivial_ln:
            lng_sb = consts.tile([PT, NT], f32, tag="lng")
            nc.sync.dma_start(out=lng_sb[:], in_=lng_d[:, :])
            lnb_sb = consts.tile([PT, NT], f32, tag="lnb")
            nc.sync.dma_start(out=lnb_sb[:], in_=lnb_d[:, :])
        if not trivial_outb:
            outb_sb = consts.tile([PT, NT], f32, tag="outb")
            nc.sync.dma_start(out=outb_sb[:], in_=outb_d[:, :])

        def emit_mm1(sc):
            xt_now = xt_store[sc]
            heads_tiles = []
            for hk in range(NT):
                ps = ps_h.tile([PT, CH], f32)
                for dt_ in range(NT):
                    nc.tensor.matmul(
                        ps[:],
                        lhsT=wp_sb[dt_][:, hk * PT:(hk + 1) * PT],
                        rhs=xt_now[dt_],
                        start=(dt_ == 0), stop=(dt_ == NT - 1),
                    )
                ht = heads_pool.tile([PT, CH], f32r)
                nc.scalar.copy(ht[:], ps[:])
                htb = headsb_pool.tile([PT, CH], bf16)
                nc.scalar.copy(htb[:], ps[:])
                heads_tiles.append((ht, htb))
            return xt_now, heads_tiles

        def emit_mm2(sc, xt_now, heads_tiles):
            cols = slice(sc * CH, (sc + 1) * CH)
            ybig_tiles = []
            for et in range(NT):
                pa = ps_a.tile([PT, CH], f32)
                pb = ps_b.tile([PT, CH], f32)
                for ft in range(NT):
                    nc.tensor.matmul(
                        pa[:],
                        lhsT=ow_sb[ft][:, et * PT:(et + 1) * PT],
                        rhs=heads_tiles[ft][0][:],
                        start=(ft == 0), stop=(ft == NT - 1),
                    )
                for ft in range(NT):
                    nc.tensor.matmul(
                        pb[:],
                        lhsT=w2_sb[ft][:, et * PT:(et + 1) * PT],
                        rhs=heads_tiles[ft][1][:],
                        start=(ft == 0), stop=(ft == NT - 1),
                    )
                yb = ybig_pool.tile([PT, 2 * CH], f32r)
                tmp = tuv_pool.tile([PT, CH], f32)
                nc.vector.tensor_mul(tmp[:], pb[:], scale_bc[:, cols])
                nc.vector.tensor_add(tmp[:], tmp[:], pa[:])
                if not trivial_outb:
                    nc.scalar.activation(tmp[:], tmp[:], AF.Identity,
                                         bias=outb_sb[:, et:et + 1], scale=1.0)
                nc.gpsimd.tensor_add(yb[:, 0:CH], tmp[:], xt_now[et].bitcast(f32))
                nc.gpsimd.tensor_mul(yb[:, CH:2 * CH], yb[:, 0:CH].bitcast(f32),
                                     yb[:, 0:CH].bitcast(f32))
                ybig_tiles.append(yb)
            return ybig_tiles

        def emit_stats_apply(sc, ybig_tiles):
            cols = slice(sc * CH, (sc + 1) * CH)
            # stats: pstat[0, :CH]=sum(y) over e, [0, CH:]=sum(y^2)
            pstat = ps_st.tile([1, 2 * CH], f32)
            for et in range(NT):
                nc.tensor.matmul(
                    pstat[:],
                    lhsT=ones_col[:],
                    rhs=ybig_tiles[et][:],
                    start=(et == 0), stop=(et == NT - 1),
                )
            # mur = [mu | rstd] rows, written as f32r so PE can broadcast
            mur = row_pool.tile([1, 2 * CH], f32r)
            nc.scalar.mul(mur[:, 0:CH], pstat[:, 0:CH], 1.0 / D)
            var = row_pool.tile([1, CH], f32)
            nc.scalar.mul(var[:], pstat[:, CH:2 * CH], 1.0 / D)
            musq = row_pool.tile([1, CH], f32)
            nc.vector.tensor_mul(musq[:], mur[:, 0:CH].bitcast(f32),
                                 mur[:, 0:CH].bitcast(f32))
            nc.vector.tensor_sub(var[:], var[:], musq[:])
            # rstd = exp(-0.5*ln(var+eps)) - stays on ACT, no DVE reciprocal
            nc.scalar.activation(var[:], var[:], AF.Ln, bias=eps_sb[:], scale=1.0)
            nc.scalar.activation(mur[:, CH:2 * CH], var[:], AF.Exp,
                                 bias=0.0, scale=-0.5)
            # broadcast down partitions via PE outer product (PE idles here)
            pbc = ps_bc.tile([PT, 2 * CH], f32)
            nc.tensor.matmul(pbc[:], lhsT=ones_row[:], rhs=mur[:],
                             start=True, stop=True)
            bc = bc_pool.tile([PT, 2 * CH], f32)
            nc.scalar.copy(bc[:], pbc[:])
            # apply LN (+ g/b unless trivial) and write out; split the
            # elementwise work across DVE and POOL so the tail drains fast
            for et in range(NT):
                z = tuv_pool.tile([PT, CH], f32)
                nc.vector.tensor_sub(z[:], ybig_tiles[et][:, 0:CH].bitcast(f32),
                                     bc[:, 0:CH])
                o = outp_pool.tile([PT, CH], f32)
                if trivial_ln:
                    nc.vector.tensor_mul(o[:], z[:], bc[:, CH:2 * CH])
                else:
                    nc.vector.tensor_mul(z[:], z[:], bc[:, CH:2 * CH])
                    nc.scalar.activation(o[:], z[:], AF.Identity,
                                         bias=lnb_sb[:, et:et + 1],
                                         scale=lng_sb[:, et:et + 1])
                qdma(out=out_d[et * PT:(et + 1) * PT, cols], in_=o[:])

        # ---- software-pipelined chunk loop: stats/apply lag one chunk,
        # x pair prefetch leads two pairs (pair 0 preloaded above) ----
        loaded = [0]

        def ensure_pairs(upto):
            while loaded[0] < min(upto, NCH // 2 - 1):
                loaded[0] += 1
                load_xt_pair(loaded[0])

        pending = None
        for sc in range(NCH):
            xt_now, heads_tiles = emit_mm1(sc)
            ensure_pairs(sc // 2 + 1)
            if pending is not None:
                emit_stats_apply(sc - 1, pending)
            pending = emit_mm2(sc, xt_now, heads_tiles)
        emit_stats_apply(NCH - 1, pending)

    nc.finalize()
    return nc


def _get_program(flags=(True, True)):
    key = ("nc", flags)
    if key not in _CACHE:
        _CACHE[key] = _build_program(flags)
    return _CACHE[key]


def make_in_maps(x, W_proj, resonance_freqs, pol_W, pol_b, imp_W1, imp_b1,
                 imp_W2, imp_b2, out_W, out_b, ln_g, ln_b, causal):
    """Host-side preprocessing: impedance path + weight folding + sharding."""
    f8 = np.float64
    impedance, coef = _host_impedance(
        x, W_proj, resonance_freqs, pol_W, pol_b, imp_W1, imp_b1, imp_W2, imp_b2)

    cosv = np.cos(resonance_freqs.astype(f8) * np.pi)
    WpT = np.ascontiguousarray(
        (W_proj.astype(f8).reshape(H * K, D) * cosv.reshape(H * K, 1)).T
    ).astype(np.float32)                                     # [d, hk]
    OWT = np.ascontiguousarray(out_W.T).astype(np.float32)   # [f, e]
    OW3 = out_W.astype(f8).reshape(D, H, K)                  # [e, i, k]
    import ml_dtypes
    W2T = [np.ascontiguousarray(
        np.einsum("ij,eik->jke", coef[b], OW3).reshape(H * K, D)
    ).astype(ml_dtypes.bfloat16) for b in range(B)]          # [jk, e] (bf16)

    causal_v = int(np.asarray(causal).item()) if np.ndim(causal) == 0 else int(causal)
    if causal_v and S > 1:
        scale_full = ((np.arange(S, dtype=f8) + 1.0) / S).astype(np.float32)
    else:
        scale_full = np.ones(S, np.float32)

    pack = lambda v: np.ascontiguousarray(
        np.asarray(v, np.float32).reshape(NT, PT).T)         # (128, 8)
    lng_p, lnb_p, outb_p = pack(ln_g), pack(ln_b), pack(out_b)

    in_maps = []
    for c in range(NCORES):
        b, hf = divmod(c, 2)
        sl = slice(hf * SC, (hf + 1) * SC)
        in_maps.append({
            "xT": np.ascontiguousarray(x[b, sl].T).astype(np.float32),
            "wpT": WpT, "owT": OWT, "w2T": W2T[b],
            "ones_col": np.ones((PT, 1), np.float32),
            "ones_row": np.ones((1, PT), np.float32),
            "scale_row": np.ascontiguousarray(scale_full[sl]),
            "lng": lng_p, "lnb": lnb_p, "outb": outb_p,
        })
    flags = (bool(np.all(np.asarray(out_b) == 0.0)),
             bool(np.all(np.asarray(ln_g) == 1.0) and np.all(np.asarray(ln_b) == 0.0)))
    return in_maps, impedance, flags


def assemble_output(results, impedance):
    normed = np.empty((B, S, D), np.float32)
    for c in range(NCORES):
        b, hf = divmod(c, 2)
        normed[b, hf * SC:(hf + 1) * SC, :] = results[c]["outT"].T
    return normed, impedance


def kernel(**inputs):
    from concourse.bass_utils import run_bass_kernel_spmd

    in_maps, impedance, flags = make_in_maps(**inputs)
    nc = _get_program(flags)
    res = run_bass_kernel_spmd(nc, in_maps, list(range(NCORES)))
    return assemble_output(res.results, impedance)


# revision 62
# speedup vs baseline: 1.0274x; 1.0274x over previous
"""Trainium2 Bass kernel for nn_DongTaiBaGuaZhen (8-core SPMD).

Sharding: core c handles (batch b = c//2, sequence half c%2) -> 2048 tokens.
The tiny cross-head impedance path (B*H*H MLP, ~0.01% of FLOPs) depends only
on mean_s(x) because the head projection is linear; it is computed on the
host and folded into a per-batch weight matrix W2_b so the device work is
three dense 1024x1024 matmuls per token chunk plus layernorm:

    heads_T = WpT.T @ x_T            (cos-modulated projection, folded on host)
    y_T     = OWT.T @ heads_T + scale(s) * (W2T.T @ heads_T) + x_T + out_b
    out_T   = LN(y_T) * g + b        (stats via PE ones-reductions)

Everything on device runs in transposed layout [features, tokens] so all
matmul contractions live on the partition axis; fp32r matmuls (1 cyc/row).
"""

import math
from contextlib import ExitStack

import numpy as np

B, S, D, H, P = 4, 4096, 1024, 8, 32
K = D // H           # 128
NCORES = 8
SC = S // 2          # tokens per core = 2048
CH = 256             # token chunk
NCH = SC // CH       # 8
PT = 128             # partitions
NT = D // PT         # 8 feature tiles

_CACHE = {}


def _erf(v):
    try:
        from scipy.special import erf
        return erf(v)
    except ImportError:
        return np.vectorize(math.erf)(v)


def _host_impedance(x, W_proj, resonance_freqs, pol_W, pol_b, imp_W1, imp_b1,
                    imp_W2, imp_b2):
    """Impedance path in float64 on host. Returns (impedance f32 (B,H,H), coef f64)."""
    f8 = np.float64
    cosv = np.cos(resonance_freqs.astype(f8) * np.pi)            # (H, K)
    xbar = x.mean(axis=1, dtype=f8)                              # (B, D)
    summary = np.einsum("bd,hkd->bhk", xbar, W_proj.astype(f8)) * cosv[None]
    pol = np.tanh(np.einsum("bhk,hpk->bhp", summary, pol_W.astype(f8))
                  + pol_b.astype(f8)[None])
    pol = pol / np.maximum(np.linalg.norm(pol, axis=-1, keepdims=True), 1e-12)
    dots = np.einsum("bhp,bgp->bhg", pol, pol)
    z = dots[..., None]
    pre = z * imp_W1.astype(f8)[:, 0] + imp_b1.astype(f8)
    hid = 0.5 * pre * (1.0 + _erf(pre / math.sqrt(2.0)))         # exact gelu
    imp = np.einsum("bijc,oc->bijo", hid, imp_W2.astype(f8))[..., 0] \
        + imp_b2.astype(f8)[0]
    imp = np.logaddexp(imp, 0.0)                                 # softplus
    eye = np.eye(H)
    impedance = imp * (1.0 - eye)
    coef = (0.1 / (1.0 + impedance)) * (1.0 - eye)
    return impedance.astype(np.float32), coef


def _build_program(flags=(True, True)):
    import concourse.bass as bass
    import concourse.bacc as bacc
    import concourse.tile as tile
    from concourse import mybir

    f32 = mybir.dt.float32
    f32r = mybir.dt.float32r
    bf16 = mybir.dt.bfloat16
    AF = mybir.ActivationFunctionType

    nc = bacc.Bacc()

    # matmul operands live as float32r end-to-end (fp32 bits; PE rounds)
    xT_d = nc.declare_dram_parameter("xT", [D, SC], f32r, isOutput=False)
    wp_d = nc.declare_dram_parameter("wpT", [D, D], f32r, isOutput=False)
    ow_d = nc.declare_dram_parameter("owT", [D, D], f32r, isOutput=False)
    w2_d = nc.declare_dram_parameter("w2T", [D, D], bf16, isOutput=False)
    ones_d = nc.declare_dram_parameter("ones_col", [PT, 1], f32r, isOutput=False)
    sr_d = nc.declare_dram_parameter("scale_row", [SC], f32, isOutput=False)
    lng_d = nc.declare_dram_parameter("lng", [PT, NT], f32, isOutput=False)
    lnb_d = nc.declare_dram_parameter("lnb", [PT, NT], f32, isOutput=False)
    outb_d = nc.declare_dram_parameter("outb", [PT, NT], f32, isOutput=False)
    out_d = nc.declare_dram_parameter("outT", [D, SC], f32, isOutput=True)

    trivial_outb, trivial_ln = flags

    with tile.TileContext(nc) as tc, ExitStack() as ctx:
        consts = ctx.enter_context(tc.tile_pool(name="consts", bufs=1))
        wpool = ctx.enter_context(tc.tile_pool(name="weights", bufs=1))
        xt_pool = ctx.enter_context(tc.tile_pool(name="xt", bufs=12))
        heads_pool = ctx.enter_context(tc.tile_pool(name="heads", bufs=10))
        headsb_pool = ctx.enter_context(tc.tile_pool(name="headsb", bufs=10))
        ybig_pool = ctx.enter_context(tc.tile_pool(name="ybig", bufs=18))
        tuv_pool = ctx.enter_context(tc.tile_pool(name="tuv", bufs=4))
        outp_pool = ctx.enter_context(tc.tile_pool(name="outp", bufs=4))
        bc_pool = ctx.enter_context(tc.tile_pool(name="bc", bufs=2))
        row_pool = ctx.enter_context(tc.tile_pool(name="rows", bufs=4))
        ps_h = ctx.enter_context(tc.tile_pool(name="ps_h", bufs=2, space="PSUM"))
        ps_a = ctx.enter_context(tc.tile_pool(name="ps_a", bufs=2, space="PSUM"))
        ps_b = ctx.enter_context(tc.tile_pool(name="ps_b", bufs=2, space="PSUM"))
        ps_st = ctx.enter_context(tc.tile_pool(name="ps_st", bufs=2, space="PSUM"))
        murd_pool = ctx.enter_context(tc.tile_pool(name="murd", bufs=2, space="DRAM"))

        # ---- DMA queue load-balancing: the two HWDGE queues (sync, scalar)
        # each sustain a limited rate, so alternate every large transfer
        # between them, emitted in priority order.
        _qs = [nc.sync, nc.scalar]
        _qi = [0]

        def qdma(**kw):
            eng = _qs[_qi[0] % len(_qs)]
            _qi[0] += 1
            eng.dma_start(**kw)

        xt_store = {}

        def load_xt_pair(p):
            tiles = []
            pcols = slice(2 * p * CH, (2 * p + 2) * CH)
            for dt_ in range(NT):
                xt = xt_pool.tile([PT, 2 * CH], f32r)
                qdma(out=xt[:], in_=xT_d[dt_ * PT:(dt_ + 1) * PT, pcols])
                tiles.append(xt)
            xt_store[2 * p] = [t[:, 0:CH] for t in tiles]
            xt_store[2 * p + 1] = [t[:, CH:2 * CH] for t in tiles]

        # startup: wp + x(pair 0) first, interleaved across both queues
        wp_sb = []
        for i in range(NT):
            t = wpool.tile([PT, D], f32r, tag=f"wp{i}")
            qdma(out=t[:], in_=wp_d[i * PT:(i + 1) * PT, :])
            wp_sb.append(t)
        load_xt_pair(0)

        ones_col = consts.tile([PT, 1], f32r, tag="ones_col")
        nc.sync.dma_start(out=ones_col[:], in_=ones_d[:, :])
        eps_sb = consts.tile([1, 1], f32, tag="eps")
        nc.vector.memset(eps_sb[:], 1e-5)
        # scale broadcast tile [128, SC] via partition-step-0 DMA from DRAM
        scale_bc = consts.tile([PT, SC], f32, tag="scale_bc")
        srow_ap = sr_d[:]
        bcast_in = bass.AP(tensor=srow_ap.tensor, offset=srow_ap.offset,
                           ap=[[0, PT]] + list(srow_ap.ap))
        nc.gpsimd.dma_start(out=scale_bc[:], in_=bcast_in)

        ow_sb, w2_sb = [], []
        for i in range(NT):
            t = wpool.tile([PT, D], f32r, tag=f"ow{i}")
            qdma(out=t[:], in_=ow_d[i * PT:(i + 1) * PT, :])
            ow_sb.append(t)
        for i in range(NT):
            t = wpool.tile([PT, D], bf16, tag=f"w2{i}")
            qdma(out=t[:], in_=w2_d[i * PT:(i + 1) * PT, :])
            w2_sb.append(t)

        if not trivial_ln:
            lng_sb = consts.tile([PT, NT], f32, tag="lng")
            nc.sync.dma_start(out=lng_sb[:], in_=lng_d[:, :])
            lnb_sb = consts.tile([PT, NT], f32, tag="lnb")
            nc.sync.dma_start(out=lnb_sb[:], in_=lnb_d[:, :])
        if not trivial_outb:
            outb_sb = consts.tile([PT, NT], f32, tag="outb")
            nc.sync.dma_start(out=outb_sb[:], in_=outb_d[:, :])

        def emit_mm1(sc, pair_tiles):
            # writes heads for chunk sc into its half of the pair tiles
            xt_now = xt_store[sc]
            half = slice(0, CH) if sc % 2 == 0 else slice(CH, 2 * CH)
            for hk in range(NT):
                ps = ps_h.tile([PT, CH], f32)
                for dt_ in range(NT):
                    nc.tensor.matmul(
                        ps[:],
                        lhsT=wp_sb[dt_][:, hk * PT:(hk + 1) * PT],
                        rhs=xt_now[dt_],
                        start=(dt_ == 0), stop=(dt_ == NT - 1),
                    )
                ht, htb = pair_tiles[hk]
                nc.scalar.copy(ht[:, half], ps[:])
                nc.scalar.copy(htb[:, half], ps[:])

        def emit_mm2_pair(p, pair_tiles):
            # one N=512 matmul pass over both chunks of the pair
            ybigs = []
            for et in range(NT):
                pa = ps_a.tile([PT, 2 * CH], f32)
                pb = ps_b.tile([PT, 2 * CH], f32)
                for ft in range(NT):
                    nc.tensor.matmul(
                        pa[:],
                        lhsT=ow_sb[ft][:, et * PT:(et + 1) * PT],
                        rhs=pair_tiles[ft][0][:],
                        start=(ft == 0), stop=(ft == NT - 1),
                    )
                for ft in range(NT):
                    nc.tensor.matmul(
                        pb[:],
                        lhsT=w2_sb[ft][:, et * PT:(et + 1) * PT],
                        rhs=pair_tiles[ft][1][:],
                        start=(ft == 0), stop=(ft == NT - 1),
                    )
                for h in range(2):
                    sc = 2 * p + h
                    if et == 0:
                        ybigs.append([])
                    cols = slice(sc * CH, (sc + 1) * CH)
                    hs = slice(h * CH, (h + 1) * CH)
                    yb = ybig_pool.tile([PT, 2 * CH], f32r)
                    tmp = tuv_pool.tile([PT, CH], f32)
                    nc.vector.tensor_mul(tmp[:], pb[:, hs], scale_bc[:, cols])
                    nc.vector.tensor_add(tmp[:], tmp[:], pa[:, hs])
                    if not trivial_outb:
                        nc.scalar.activation(tmp[:], tmp[:], AF.Identity,
                                             bias=outb_sb[:, et:et + 1], scale=1.0)
                    nc.gpsimd.tensor_add(yb[:, 0:CH], tmp[:],
                                         xt_store[sc][et].bitcast(f32))
                    nc.gpsimd.tensor_mul(yb[:, CH:2 * CH], yb[:, 0:CH].bitcast(f32),
                                         yb[:, 0:CH].bitcast(f32))
                    ybigs[h].append(yb)
            return ybigs

        def emit_stats_apply(sc, ybig_tiles):
            cols = slice(sc * CH, (sc + 1) * CH)
            # stats: pstat[0, :CH]=sum(y) over e, [0, CH:]=sum(y^2)
            pstat = ps_st.tile([1, 2 * CH], f32)
            for et in range(NT):
                nc.tensor.matmul(
                    pstat[:],
                    lhsT=ones_col[:],
                    rhs=ybig_tiles[et][:],
                    start=(et == 0), stop=(et == NT - 1),
                )
            # mur = [mu | rstd]
            mur = row_pool.tile([1, 2 * CH], f32)
            nc.scalar.mul(mur[:, 0:CH], pstat[:, 0:CH], 1.0 / D)
            nc.scalar.mul(mur[:, CH:2 * CH], pstat[:, CH:2 * CH], 1.0 / D)
            musq = row_pool.tile([1, CH], f32)
            nc.vector.tensor_mul(musq[:], mur[:, 0:CH], mur[:, 0:CH])
            nc.vector.tensor_sub(mur[:, CH:2 * CH], mur[:, CH:2 * CH], musq[:])
            nc.scalar.activation(mur[:, CH:2 * CH], mur[:, CH:2 * CH], AF.Sqrt,
                                 bias=eps_sb[:], scale=1.0)
            nc.vector.reciprocal(mur[:, CH:2 * CH], mur[:, CH:2 * CH])
            # broadcast down partitions: DRAM bounce + partition-step-0 DMA
            murd = murd_pool.tile([1, 2 * CH], f32)
            nc.sync.dma_start(out=murd[:], in_=mur[:])
            murd_ap = murd[:]
            bc_in = bass.AP(tensor=murd_ap.tensor, offset=murd_ap.offset,
                            ap=[[0, PT]] + list(murd_ap.ap)[1:])
            bc = bc_pool.tile([PT, 2 * CH], f32)
            nc.gpsimd.dma_start(out=bc[:], in_=bc_in)
            # apply LN (+ g/b unless trivial) and write out; split the
            # elementwise work across DVE and POOL so the tail drains fast
            for et in range(NT):
                z = tuv_pool.tile([PT, CH], f32)
                nc.vector.tensor_sub(z[:], ybig_tiles[et][:, 0:CH].bitcast(f32),
                                     bc[:, 0:CH])
                o = outp_pool.tile([PT, CH], f32)
                if trivial_ln:
                    nc.vector.tensor_mul(o[:], z[:], bc[:, CH:2 * CH])
                else:
                    nc.vector.tensor_mul(z[:], z[:], bc[:, CH:2 * CH])
                    nc.scalar.activation(o[:], z[:], AF.Identity,
                                         bias=lnb_sb[:, et:et + 1],
                                         scale=lng_sb[:, et:et + 1])
                qdma(out=out_d[et * PT:(et + 1) * PT, cols], in_=o[:])

        # ---- software-pipelined chunk loop: stats/apply lag one chunk,
        # x pair prefetch leads two pairs (pair 0 preloaded above) ----
        loaded = [0]

        def ensure_pairs(upto):
            while loaded[0] < min(upto, NCH // 2 - 1):
                loaded[0] += 1
                load_xt_pair(loaded[0])

        pend = []
        for p in range(NCH // 2):
            pair_tiles = [(heads_pool.tile([PT, 2 * CH], f32r, tag="hp",
                                           name=f"hp{p}_{i}"),
                           headsb_pool.tile([PT, 2 * CH], bf16, tag="hbp",
                                            name=f"hbp{p}_{i}"))
                          for i in range(NT)]
            emit_mm1(2 * p, pair_tiles)
            ensure_pairs(p + 1)
            if pend:
                emit_stats_apply(*pend.pop(0))
            emit_mm1(2 * p + 1, pair_tiles)
            if pend:
                emit_stats_apply(*pend.pop(0))
            yb0, yb1 = emit_mm2_pair(p, pair_tiles)
            pend.extend([(2 * p, yb0), (2 * p + 1, yb1)])
        for it in pend:
            emit_stats_apply(*it)

    nc.finalize()
    return nc


def _get_program(flags=(True, True)):
    key = ("nc", flags)
    if key not in _CACHE:
        _CACHE[key] = _build_program(flags)
    return _CACHE[key]


def make_in_maps(x, W_proj, resonance_freqs, pol_W, pol_b, imp_W1, imp_b1,
                 imp_W2, imp_b2, out_W, out_b, ln_g, ln_b, causal):
    """Host-side preprocessing: impedance path + weight folding + sharding."""
    f8 = np.float64
    impedance, coef = _host_impedance(
        x, W_proj, resonance_freqs, pol_W, pol_b, imp_W1, imp_b1, imp_W2, imp_b2)

    cosv = np.cos(resonance_freqs.astype(f8) * np.pi)
    WpT = np.ascontiguousarray(
        (W_proj.astype(f8).reshape(H * K, D) * cosv.reshape(H * K, 1)).T
    ).astype(np.float32)                                     # [d, hk]
    OWT = np.ascontiguousarray(out_W.T).astype(np.float32)   # [f, e]
    OW3 = out_W.astype(f8).reshape(D, H, K)                  # [e, i, k]
    import ml_dtypes
    W2T = [np.ascontiguousarray(
        np.einsum("ij,eik->jke", coef[b], OW3).reshape(H * K, D)
    ).astype(ml_dtypes.bfloat16) for b in range(B)]          # [jk, e] (bf16)

    causal_v = int(np.asarray(causal).item()) if np.ndim(causal) == 0 else int(causal)
    if causal_v and S > 1:
        scale_full = ((np.arange(S, dtype=f8) + 1.0) / S).astype(np.float32)
    else:
        scale_full = np.ones(S, np.float32)

    pack = lambda v: np.ascontiguousarray(
        np.asarray(v, np.float32).reshape(NT, PT).T)         # (128, 8)
    lng_p, lnb_p, outb_p = pack(ln_g), pack(ln_b), pack(out_b)

    in_maps = []
    for c in range(NCORES):
        b, hf = divmod(c, 2)
        sl = slice(hf * SC, (hf + 1) * SC)
        in_maps.append({
            "xT": np.ascontiguousarray(x[b, sl].T).astype(np.float32),
            "wpT": WpT, "owT": OWT, "w2T": W2T[b],
            "ones_col": np.ones((PT, 1), np.float32),
            "scale_row": np.ascontiguousarray(scale_full[sl]),
            "lng": lng_p, "lnb": lnb_p, "outb": outb_p,
        })
    flags = (bool(np.all(np.asarray(out_b) == 0.0)),
             bool(np.all(np.asarray(ln_g) == 1.0) and np.all(np.asarray(ln_b) == 0.0)))
    return in_maps, impedance, flags


def assemble_output(results, impedance):
    normed = np.empty((B, S, D), np.float32)
    for c in range(NCORES):
        b, hf = divmod(c, 2)
        normed[b, hf * SC:(hf + 1) * SC, :] = results[c]["outT"].T
    return normed, impedance


def kernel(**inputs):
    from concourse.bass_utils import run_bass_kernel_spmd

    in_maps, impedance, flags = make_in_maps(**inputs)
    nc = _get_program(flags)
    res = run_bass_kernel_spmd(nc, in_maps, list(range(NCORES)))
    return assemble_output(res.results, impedance)
